# revision 16
# baseline (speedup 1.0000x reference)
"""Trainium2 Bass kernel for 16-head MHA (B=2, S=2048, D=1024, E=64).

Sharding: 8 cores = 2 batches x 4 head-groups. Each core computes 4 heads
(2 pairs of 2) for one batch and returns a partial output [2048, 1024]
(sum of its 4 heads' contributions after the output projection). Host sums
the 4 partials per batch.

Design: the Scalar engine (ACT) is the hard floor — 16.8M softmax exps at
1 elem/cycle/lane = ~147us/core. Everything else is scheduled to hide
under the exp stream:
  - inputs arrive via 3D-AP batched DMAs in query-chunk-sized pieces,
    highest-priority first (xk t0, xq t0), so the first S^T matmul and
    exp issue ~15us into the kernel
  - V is projected TOKEN-major directly on the PE (lhsT = x^T tile,
    rhs = W_val 4-head block) — no DMA transpose
  - V projection + remaining K/Q chunks interleave into the attention
    stream in <=8-matmul bites so the S^T cadence (= exp cadence) holds
  - all PSUM evacuations run on DVE, never ACT
  - output projection of chunk qc interleaves into chunk qc+1's stream;
    for the last chunk the p0 half is pre-computed into SBUF and only
    the p1 half + add remains after the final exp
"""

import sys

sys.path.insert(0, "/opt/trn_rl_repo")

import numpy as np

import concourse.bass as bass
import concourse.bacc as bacc
import concourse.mybir as mybir
from concourse import tile
from concourse.tile_rust import add_dep_helper
from concourse.bass_interp import get_hw_module
from concourse.bass_utils import run_bass_kernel_spmd

F16 = mybir.dt.float16
F32 = mybir.dt.float32
BF16 = mybir.dt.bfloat16

N_CORES = 8
T = 2048          # tokens per core (one batch)
D = 1024          # model dim
E = 64            # head dim
QC = 512          # query chunk
NQ = T // QC      # 4 query chunks
KB = 128          # key block
NKB = T // KB     # 16 key blocks
ND = D // 128     # 8 contraction chunks for projections

_CACHE = {}


def _build():
    nc = bacc.Bacc("TRN2", target_bir_lowering=False, debug=False,
                   num_devices=N_CORES)

    xqT = nc.dram_tensor("xqT", [D, T], F16, kind="ExternalInput").ap()
    xkT = nc.dram_tensor("xkT", [D, T], F16, kind="ExternalInput").ap()
    xvT = nc.dram_tensor("xvT", [D, T], F16, kind="ExternalInput").ap()
    # per-pair packed weights, layout [128, 8*128]: chunk d at cols d*128
    wq = [nc.dram_tensor(f"wq{p}", [128, D], F16, kind="ExternalInput").ap()
          for p in range(2)]
    wk = [nc.dram_tensor(f"wk{p}", [128, D], F16, kind="ExternalInput").ap()
          for p in range(2)]
    # wv: chunk d at cols d*256; within a chunk, head idx at cols idx*64
    wv = nc.dram_tensor("wv", [128, 2 * D], F16, kind="ExternalInput").ap()
    wo = [nc.dram_tensor(f"wo{p}", [128, D], F16, kind="ExternalInput").ap()
          for p in range(2)]
    pout = nc.dram_tensor("pout", [T, D], F16, kind="ExternalOutput").ap()

    with tile.TileContext(nc) as tc:
        with (
            tc.tile_pool(name="consts", bufs=1) as consts,
            tc.tile_pool(name="persist", bufs=1) as persist,
            tc.tile_pool(name="xs", bufs=1) as xsp,
            tc.tile_pool(name="at", bufs=6) as atp,
            tc.tile_pool(name="o2t", bufs=2) as o2tp,
            tc.tile_pool(name="os", bufs=2) as osp,
            tc.tile_pool(name="small", bufs=2) as smallp,
            tc.tile_pool(name="psS", bufs=2, space="PSUM") as psS,
            tc.tile_pool(name="psO", bufs=1, space="PSUM") as psO,
            tc.tile_pool(name="psP", bufs=2, space="PSUM") as psP,
        ):
            # ---- constants ----
            wq_sb = [consts.tile([128, D], F16, tag=f"wq{p}", name=f"wq_sb{p}") for p in range(2)]
            wk_sb = [consts.tile([128, D], F16, tag=f"wk{p}", name=f"wk_sb{p}") for p in range(2)]
            wo_sb = [consts.tile([128, D], F16, tag=f"wo{p}", name=f"wo_sb{p}") for p in range(2)]
            wv_sb = consts.tile([128, 2 * D], F16, tag="wv", name="wv_sb")

            # exp table preload: tiny dummy activation fires the table DMA
            # during the input-load window instead of before the first real exp
            dum_in = consts.tile([1, 8], F32, tag="dumi", name="dum_in")
            dum_out = consts.tile([1, 8], BF16, tag="dumo", name="dum_out")
            nc.vector.memset(dum_in[:], 0.0)
            nc.scalar.activation(dum_out[:], dum_in[:],
                                 mybir.ActivationFunctionType.Exp)
            junk = consts.tile([128, 577], F16, tag="junk", name="junk")
            nc.vector.memset(junk[:], 0.25)

            # ---- input tiles + DMA (priority order) ----
            # one [128, 8*512] tile per query-chunk-width slice per tensor;
            # cols d*512+c hold d-chunk d. One batched 3D-AP DMA per slice.
            xk_q = [xsp.tile([128, ND * QC], F16, tag=f"xk{t}", name=f"xk{t}")
                    for t in range(1, NQ)]
            xk_q = [None] + xk_q
            xk0h = [xsp.tile([128, ND * 256], F16, tag=f"xk0{h}", name=f"xk0{h}")
                    for h in range(2)]
            xq_q = [xsp.tile([128, ND * QC], F16, tag=f"xq{t}", name=f"xq{t}")
                    for t in range(NQ)]
            xv_q = [xsp.tile([128, ND * QC], F16, tag=f"xv{t}", name=f"xv{t}")
                    for t in range(NQ)]

            def dma_slice(dst, srcT, t):
                nc.sync.dma_start(
                    dst[:].rearrange("p (d c) -> p d c", d=ND),
                    srcT[:, t * QC:(t + 1) * QC]
                    .rearrange("(d p) c -> p d c", p=128))

            def dma_slice_h(dst, srcT, c0):
                nc.sync.dma_start(
                    dst[:].rearrange("p (d c) -> p d c", d=ND),
                    srcT[:, c0:c0 + 256]
                    .rearrange("(d p) c -> p d c", p=128))

            dma_slice_h(xk0h[0], xkT, 0)
            nc.sync.dma_start(wk_sb[0][:], wk[0][:])
            dma_slice(xq_q[0], xqT, 0)
            nc.sync.dma_start(wk_sb[1][:], wk[1][:])
            dma_slice_h(xk0h[1], xkT, 256)
            for p in range(2):
                nc.sync.dma_start(wq_sb[p][:], wq[p][:])
            nc.sync.dma_start(wv_sb[:], wv[:])
            dma_slice(xv_q[0], xvT, 0)
            dma_slice(xk_q[1], xkT, 1)
            dma_slice(xv_q[1], xvT, 1)
            dma_slice(xk_q[2], xkT, 2)
            dma_slice(xv_q[2], xvT, 2)
            dma_slice(xk_q[3], xkT, 3)
            dma_slice(xq_q[1], xqT, 1)
            dma_slice(xv_q[3], xvT, 3)
            dma_slice(xq_q[2], xqT, 2)
            dma_slice(xq_q[3], xqT, 3)
            for p in range(2):
                nc.sync.dma_start(wo_sb[p][:], wo[p][:])

            # ---- persistent activations ----
            kt = [persist.tile([128, T], F16, tag=f"kt{p}", name=f"kt{p}") for p in range(2)]
            qt = [[persist.tile([128, QC], F16, tag=f"qt{p}_{t}", name=f"qt{p}_{t}")
                   for t in range(NQ)] for p in range(2)]
            # token-major [V|1] per key-block: cols idx*65..idx*65+63 = V of
            # head idx (=2p+h), col idx*65+64 = ones (fused row-sum)
            v2 = [persist.tile([128, 4 * 65], BF16, tag=f"v2_{b}", name=f"v2_{b}")
                  for b in range(NKB)]
            for b in range(NKB):
                for idx in range(4):
                    nc.vector.memset(v2[b][:, idx * 65 + 64: idx * 65 + 65], 1.0)

            # ---- projection emitters ----
            def proj_full(w_sb_p, x_q, dst_ap):
                """one [128, 512] projection slice in one go"""
                ps = psP.tile([128, QC], F32, tag="pp", name="proj_ps")
                for d in range(ND):
                    nc.tensor.matmul(ps[:], w_sb_p[:, d * 128:(d + 1) * 128],
                                     x_q[:, d * QC:(d + 1) * QC],
                                     start=(d == 0), stop=(d == ND - 1))
                nc.vector.tensor_copy(dst_ap, ps[:])

            def proj_chunks(w_sb_p, x_q, dst_ap, splits):
                """same, split into len(splits) PE bites for interleaving"""
                state = {}
                bnds = [0]
                for s in splits:
                    bnds.append(bnds[-1] + s)
                assert bnds[-1] == ND
                out = []
                for ci in range(len(splits)):
                    lo, hi = bnds[ci], bnds[ci + 1]

                    def f(lo=lo, hi=hi):
                        if "ps" not in state:
                            state["ps"] = psP.tile([128, QC], F32, tag="pp",
                                                   name="proj_ps")
                        ps = state["ps"]
                        for d in range(lo, hi):
                            nc.tensor.matmul(
                                ps[:], w_sb_p[:, d * 128:(d + 1) * 128],
                                x_q[:, d * QC:(d + 1) * QC],
                                start=(d == 0), stop=(d == ND - 1))
                        if hi == ND:
                            nc.vector.tensor_copy(dst_ap, ps[:])
                    out.append(f)
                return out

            def kproj_t0(p, piece):
                ps = psP.tile([128, QC], F32, tag="pp", name=f"k0_{p}_{piece}")
                xt = xk0h[piece]
                for d in range(ND):
                    nc.tensor.matmul(ps[:, 0:256],
                                     wk_sb[p][:, d * 128:(d + 1) * 128],
                                     xt[:, d * 256:(d + 1) * 256],
                                     start=(d == 0), stop=(d == ND - 1))
                nc.vector.tensor_copy(
                    kt[p][:, piece * 256:(piece + 1) * 256], ps[:, 0:256])

            def v_slice(b):
                """token-major V for key-block b: all 4 heads at once."""
                tq, c0 = b // 4, (b % 4) * 128
                pv = psP.tile([128, QC], F32, tag="pp", name=f"v_ps{b}")
                for d in range(ND):
                    nc.tensor.matmul(pv[:, 0:256],
                                     xv_q[tq][:, d * QC + c0: d * QC + c0 + 128],
                                     wv_sb[:, d * 256:(d + 1) * 256],
                                     start=(d == 0), stop=(d == ND - 1))
                for idx in range(4):
                    nc.vector.tensor_copy(
                        v2[b][:, idx * 65: idx * 65 + 64],
                        pv[:, idx * 64:(idx + 1) * 64])

            # PE warmup: garbage matmuls keep the HAM clock gate open
            # (K=8/8) until the first real projection
            for i in range(16):
                dmy = psO.tile([65, QC], F32, tag=f"o{i % 2}", name=f"wu_{i}")
                nc.tensor.matmul(dmy[:], junk[:, 512:577], junk[:, 0:QC],
                                 start=True, stop=True)

            # minimal critical path: K-p0 keys 0:256 + Q-p0 chunk 0 unlock
            # S^T kb0/kb1; everything else rides the attention stream
            kproj_t0(0, 0)
            proj_full(wq_sb[0], xq_q[0], qt[0][0][:])

            # deferred projection work, interleaved into attention groups:
            # (qc, p) -> {kb: [thunk, ...]}
            hooks = {}

            def add_hook(qc, p, kb, fn):
                hooks.setdefault((qc, p), {}).setdefault(kb, []).append(fn)

            def add_chunked(qc, p, kb0, w_sb_p, x_q, dst_ap, splits=(4, 4)):
                for i, f in enumerate(
                        proj_chunks(w_sb_p, x_q, dst_ap, splits)):
                    add_hook(qc, p, kb0 + i, f)

            # V token-major: one key-block per kb of group (0,0)
            for b in range(NKB):
                add_hook(0, 0, b, (lambda b=b: v_slice(b)))
            # K-p0 second piece + remaining p0 K slices, just before their
            # S^T deadlines inside (0,0)
            add_hook(0, 0, 0, lambda: kproj_t0(0, 1))
            for t in range(1, NQ):
                add_chunked(0, 0, 4 * t - 2, wk_sb[0], xk_q[t],
                            kt[0][:, t * QC:(t + 1) * QC])
            # Q-p1 chunk 0 and K-p1 t0 pieces late in (0,0) (needed at (0,1))
            add_chunked(0, 0, 4, wq_sb[1], xq_q[0], qt[1][0][:])
            add_hook(0, 0, 12, lambda: kproj_t0(1, 0))
            add_hook(0, 0, 13, lambda: kproj_t0(1, 1))
            # p1 K slices inside (0,1), before their S^T deadlines there
            for t in range(1, NQ):
                add_chunked(0, 1, 2 * t - 2, wk_sb[1], xk_q[t],
                            kt[1][:, t * QC:(t + 1) * QC])
            # remaining Q chunks: qt[t] needed at group (t, *); emitted in
            # outproj-free groups in 3-matmul bites
            add_chunked(0, 1, 8, wq_sb[0], xq_q[1], qt[0][1][:], (3, 3, 2))
            add_chunked(0, 1, 11, wq_sb[1], xq_q[1], qt[1][1][:], (3, 3, 2))
            for t in (2, 3):
                g = (t - 1, 0)
                add_chunked(g[0], g[1], 1, wq_sb[0], xq_q[t], qt[0][t][:],
                            (3, 3, 2))
                add_chunked(g[0], g[1], 4, wq_sb[1], xq_q[t], qt[1][t][:],
                            (3, 3, 2))

            # ---- attention + output projection ----
            ost_live = {}

            def emit_outproj_group(qc, o2t, sub, oc, anchor):
                q0 = qc * QC
                if oc == 0:
                    ost_live[(qc, sub)] = osp.tile(
                        [128, D], F16, tag="os", name=f"os_{qc}_{sub}")
                ost = ost_live[(qc, sub)]
                pp = psP.tile([128, QC], F32, tag="pp", name=f"pp_{qc}_{sub}_{oc}")
                for p in range(2):
                    mm = nc.tensor.matmul(
                        pp[:],
                        o2t[p][:, sub * 128:(sub + 1) * 128],
                        wo_sb[p][:, oc * QC:(oc + 1) * QC],
                        start=(p == 0), stop=(p == 1))
                    if p == 0 and anchor is not None:
                        add_dep_helper(mm.ins, anchor.ins, sync=False,
                                       reason="interleave outproj after S")
                nc.vector.tensor_copy(
                    ost[:, oc * QC:(oc + 1) * QC], pp[:])
                if oc == 1:
                    nc.sync.dma_start(
                        pout[q0 + sub * 128:q0 + (sub + 1) * 128, :],
                        ost[:])
                    del ost_live[(qc, sub)]

            # last-chunk p0-half of the output projection, precomputed into
            # SBUF during the (3,1) stream so only p1 remains in the tail
            s3 = {}

            def emit_outproj3_p0(o2t_p0, sub, oc, anchor):
                s3[(sub, oc)] = persist.tile([128, QC], F16,
                                             tag=f"s3_{sub}_{oc}",
                                             name=f"s3_{sub}_{oc}")
                pp = psP.tile([128, QC], F32, tag="pp", name=f"p3_{sub}_{oc}")
                mm = nc.tensor.matmul(pp[:],
                                      o2t_p0[:, sub * 128:(sub + 1) * 128],
                                      wo_sb[0][:, oc * QC:(oc + 1) * QC],
                                      start=True, stop=True)
                if anchor is not None:
                    add_dep_helper(mm.ins, anchor.ins, sync=False,
                                   reason="interleave outproj3 after S")
                nc.vector.tensor_copy(s3[(sub, oc)][:], pp[:])

            def emit_outproj3_p1(o2t_p1):
                for sub in range(4):
                    ost = osp.tile([128, D], F16, tag="os", name=f"os3_{sub}")
                    for oc in range(2):
                        pp = psP.tile([128, QC], F32, tag="pp",
                                      name=f"pf_{sub}_{oc}")
                        nc.tensor.matmul(pp[:],
                                         o2t_p1[:, sub * 128:(sub + 1) * 128],
                                         wo_sb[1][:, oc * QC:(oc + 1) * QC],
                                         start=True, stop=True)
                        nc.vector.tensor_tensor(
                            ost[:, oc * QC:(oc + 1) * QC],
                            s3[(sub, oc)][:], pp[:],
                            mybir.AluOpType.add)
                    nc.sync.dma_start(
                        pout[3 * QC + sub * 128:3 * QC + (sub + 1) * 128, :],
                        ost[:])

            pending = None
            for qc in range(NQ):
                o2t = [o2tp.tile([128, QC], F16, tag=f"o2t{p}", name=f"o2t_{qc}_{p}")
                       for p in range(2)]
                for p in range(2):
                    gh = hooks.get((qc, p), {})
                    po = [psO.tile([65, QC], F32, tag=f"o{h}", name=f"po_{qc}_{p}_{h}")
                          for h in range(2)]
                    for kb in range(NKB):
                        k0 = kb * KB
                        ps = psS.tile([128, 2 * QC], F32, tag="s", name=f"s_{qc}_{p}_{kb}")
                        s_anchor = nc.tensor.matmul(
                            ps[:, 0:QC],
                            kt[p][0:64, k0:k0 + KB],
                            qt[p][qc][0:64, :],
                            start=True, stop=True, tile_position=(0, 0))
                        nc.tensor.matmul(
                            ps[:, QC:2 * QC],
                            kt[p][64:128, k0:k0 + KB],
                            qt[p][qc][64:128, :],
                            start=True, stop=True, tile_position=(64, 0))
                        for fn in gh.get(kb, ()):
                            fn()
                        at = atp.tile([128, 2 * QC], BF16, tag="at", name=f"at_{qc}_{p}_{kb}")
                        nc.scalar.activation(
                            at[:], ps[:], mybir.ActivationFunctionType.Exp)
                        for h in range(2):
                            idx = 2 * p + h
                            nc.tensor.matmul(
                                po[h][:],
                                v2[kb][:, idx * 65: idx * 65 + 65],
                                at[:, h * QC:(h + 1) * QC],
                                start=(kb == 0), stop=(kb == NKB - 1))
                        if p == 1 and pending is not None and 6 <= kb < 14:
                            pqc, po2t = pending
                            emit_outproj_group(pqc, po2t, (kb - 6) // 2,
                                               (kb - 6) % 2, s_anchor)
                        if qc == 3 and p == 1 and kb >= 8:
                            emit_outproj3_p0(o2t[0], (kb - 8) // 2,
                                             (kb - 8) % 2, s_anchor)
                    # evacuate O^T fast first (frees PSUM for the next
                    # group's AV), then normalize off the critical path
                    ots = []
                    for h in range(2):
                        ot = smallp.tile([65, QC], F32, tag=f"ot{h}", name=f"ot_{qc}_{p}_{h}")
                        nc.vector.tensor_copy(ot[:], po[h][:])
                        ots.append(ot)
                    rsl = []
                    for h in range(2):
                        r = smallp.tile([1, QC], F32, tag=f"r{h}", name=f"r_{qc}_{p}_{h}")
                        if qc == 3 and p == 1:
                            # no more exps -> ACT is idle; 1/d = exp(-ln d)
                            # (both fns share a table set), off the DVE chain
                            lg = smallp.tile([1, QC], F32, tag=f"lg{h}",
                                             name=f"lg_{h}")
                            nc.scalar.activation(
                                lg[:], ots[h][64:65, :],
                                mybir.ActivationFunctionType.Ln)
                            nc.scalar.activation(
                                r[:], lg[:],
                                mybir.ActivationFunctionType.Exp,
                                scale=-1.0)
                        else:
                            nc.vector.reciprocal(r[:], ots[h][64:65, :])
                        rsl.append(r)
                    rbs = []
                    for h in range(2):
                        rb = smallp.tile([64, QC], F32, tag=f"rb{h}", name=f"rb_{qc}_{p}_{h}")
                        nc.gpsimd.partition_broadcast(rb[:], rsl[h][:])
                        rbs.append(rb)
                    for h in range(2):
                        nc.vector.tensor_mul(
                            o2t[p][h * 64:(h + 1) * 64, :],
                            ots[h][0:64, :], rbs[h][:])
                    if p == 1:
                        pending = None
                pending = (qc, o2t)

            # tail: PE warm-keeping dummies under the final normalize chain
            for i in range(36):
                dmy = psO.tile([65, QC], F32, tag=f"o{i % 2}", name=f"dmy_{i}")
                nc.tensor.matmul(dmy[:], kt[0][:, 0:65], kt[1][:, 0:QC],
                                 start=True, stop=True)
            emit_outproj3_p1(pending[1][1])

    nc.compile()
    nc.m = get_hw_module(nc.m)
    return nc


def _pack_w(w_pair):
    # w_pair: [2, 1024, 64] -> [1024, 128] -> chunk-major [128, 8*128]
    w = np.concatenate([w_pair[0], w_pair[1]], axis=1)          # [1024, 128]
    return np.ascontiguousarray(
        w.reshape(ND, 128, 128).transpose(1, 0, 2).reshape(128, D))


def _pack_wv(w4):
    # w4: [4, 1024, 64] -> [128, 8*256]; chunk d at cols d*256, head idx
    # (=2p+h) at idx*64 within the chunk
    return np.ascontiguousarray(
        w4.reshape(4, ND, 128, E).transpose(2, 1, 0, 3).reshape(128, 2 * D))


def _pack_wo(wo_pair):
    # wo_pair: [2, 64, 1024] -> [128, 1024]
    return np.ascontiguousarray(np.concatenate([wo_pair[0], wo_pair[1]], axis=0))


def kernel(q, k, v, W_query, W_key, W_val, W_out, _trace=False):
    q = np.asarray(q, dtype=np.float32)
    k = np.asarray(k, dtype=np.float32)
    v = np.asarray(v, dtype=np.float32)
    W_query = np.asarray(W_query, dtype=np.float32)
    W_key = np.asarray(W_key, dtype=np.float32)
    W_val = np.asarray(W_val, dtype=np.float32)
    W_out = np.asarray(W_out, dtype=np.float32)

    if "nc" not in _CACHE:
        _CACHE["nc"] = _build()
    nc = _CACHE["nc"]

    norm = 1.0 / np.sqrt(E)
    xT = {}
    for b in range(2):
        xT[("q", b)] = np.ascontiguousarray(q[b].T).astype(np.float16)
        xT[("k", b)] = np.ascontiguousarray(k[b].T).astype(np.float16)
        xT[("v", b)] = np.ascontiguousarray(v[b].T).astype(np.float16)

    in_maps = []
    for c in range(N_CORES):
        b, g = c // 4, c % 4
        hs = [4 * g, 4 * g + 1, 4 * g + 2, 4 * g + 3]
        m = {
            "xqT": xT[("q", b)], "xkT": xT[("k", b)], "xvT": xT[("v", b)],
            "wv": _pack_wv(W_val[hs]).astype(np.float16),
        }
        for p in range(2):
            hp = hs[2 * p:2 * p + 2]
            m[f"wq{p}"] = _pack_w(W_query[hp] * norm).astype(np.float16)
            m[f"wk{p}"] = _pack_w(W_key[hp]).astype(np.float16)
            m[f"wo{p}"] = _pack_wo(W_out[hp]).astype(np.float16)
        in_maps.append(m)

    res = run_bass_kernel_spmd(nc, in_maps, list(range(N_CORES)),
                               trace=_trace)
    parts = [res.results[c]["pout"].astype(np.float32) for c in range(N_CORES)]
    out = np.stack([
        parts[0] + parts[1] + parts[2] + parts[3],
        parts[4] + parts[5] + parts[6] + parts[7],
    ]).astype(np.float32)
    if _trace:
        _CACHE["last_result"] = res
    return out


# revision 18
# speedup vs baseline: 1.1348x; 1.1348x over previous
"""Trainium2 Bass kernel for 16-head MHA (B=2, S=2048, D=1024, E=64).

Sharding: 8 cores = 2 batches x 4 head-groups. Each core computes 4 heads
(2 pairs of 2) for one batch and returns a partial output [2048, 1024]
(sum of its 4 heads' contributions after the output projection). Host sums
the 4 partials per batch.

Design: the Scalar engine (ACT) is the hard floor — 16.8M softmax exps at
1 elem/cycle/lane = ~147us/core. Everything else is scheduled to hide
under the exp stream:
  - inputs arrive via 3D-AP batched DMAs in query-chunk-sized pieces,
    highest-priority first (xk t0, xq t0), so the first S^T matmul and
    exp issue ~15us into the kernel
  - V is projected TOKEN-major directly on the PE (lhsT = x^T tile,
    rhs = W_val 4-head block) — no DMA transpose
  - V projection + remaining K/Q chunks interleave into the attention
    stream in <=8-matmul bites so the S^T cadence (= exp cadence) holds
  - all PSUM evacuations run on DVE, never ACT
  - output projection of chunk qc interleaves into chunk qc+1's stream;
    for the last chunk the p0 half is pre-computed into SBUF and only
    the p1 half + add remains after the final exp
"""

import sys

sys.path.insert(0, "/opt/trn_rl_repo")

import numpy as np

import concourse.bass as bass
import concourse.bacc as bacc
import concourse.mybir as mybir
from concourse import tile
from concourse.tile_rust import add_dep_helper
from concourse.bass_interp import get_hw_module
from concourse.bass_utils import run_bass_kernel_spmd

F16 = mybir.dt.float16
F32 = mybir.dt.float32
BF16 = mybir.dt.bfloat16

N_CORES = 8
T = 2048          # tokens per core (one batch)
D = 1024          # model dim
E = 64            # head dim
QC = 512          # query chunk
NQ = T // QC      # 4 query chunks
KB = 128          # key block
NKB = T // KB     # 16 key blocks
ND = D // 128     # 8 contraction chunks for projections

_CACHE = {}


def _build():
    nc = bacc.Bacc("TRN2", target_bir_lowering=False, debug=False,
                   num_devices=N_CORES)

    xqT = nc.dram_tensor("xqT", [D, T], F16, kind="ExternalInput").ap()
    xkT = nc.dram_tensor("xkT", [D, T], F16, kind="ExternalInput").ap()
    xvT = nc.dram_tensor("xvT", [D, T], F16, kind="ExternalInput").ap()
    # per-pair packed weights, layout [128, 8*128]: chunk d at cols d*128
    wq = [nc.dram_tensor(f"wq{p}", [128, D], F16, kind="ExternalInput").ap()
          for p in range(2)]
    wk = [nc.dram_tensor(f"wk{p}", [128, D], F16, kind="ExternalInput").ap()
          for p in range(2)]
    # wv: chunk d at cols d*256; within a chunk, head idx at cols idx*64
    wv = nc.dram_tensor("wv", [128, 2 * D], F16, kind="ExternalInput").ap()
    wo = [nc.dram_tensor(f"wo{p}", [128, D], F16, kind="ExternalInput").ap()
          for p in range(2)]
    pout = nc.dram_tensor("pout", [T, D], F16, kind="ExternalOutput").ap()

    with tile.TileContext(nc) as tc:
        with (
            tc.tile_pool(name="consts", bufs=1) as consts,
            tc.tile_pool(name="persist", bufs=1) as persist,
            tc.tile_pool(name="xs", bufs=1) as xsp,
            tc.tile_pool(name="at", bufs=6) as atp,
            tc.tile_pool(name="o2t", bufs=2) as o2tp,
            tc.tile_pool(name="os", bufs=2) as osp,
            tc.tile_pool(name="small", bufs=2) as smallp,
            tc.tile_pool(name="psS", bufs=2, space="PSUM") as psS,
            tc.tile_pool(name="psO", bufs=1, space="PSUM") as psO,
            tc.tile_pool(name="psP", bufs=2, space="PSUM") as psP,
        ):
            # ---- constants ----
            wq_sb = [consts.tile([128, D], F16, tag=f"wq{p}", name=f"wq_sb{p}") for p in range(2)]
            wk_sb = [consts.tile([128, D], F16, tag=f"wk{p}", name=f"wk_sb{p}") for p in range(2)]
            wo_sb = [consts.tile([128, D], F16, tag=f"wo{p}", name=f"wo_sb{p}") for p in range(2)]
            wv_sb = consts.tile([128, 2 * D], F16, tag="wv", name="wv_sb")

            # exp table preload: tiny dummy activation fires the table DMA
            # during the input-load window instead of before the first real exp
            dum_in = consts.tile([1, 8], F32, tag="dumi", name="dum_in")
            dum_out = consts.tile([1, 8], BF16, tag="dumo", name="dum_out")
            nc.vector.memset(dum_in[:], 0.0)
            nc.scalar.activation(dum_out[:], dum_in[:],
                                 mybir.ActivationFunctionType.Exp)
            junk = consts.tile([128, 577], F16, tag="junk", name="junk")
            nc.vector.memset(junk[:], 0.25)

            # ---- input tiles + DMA (priority order) ----
            # one [128, 8*512] tile per query-chunk-width slice per tensor;
            # cols d*512+c hold d-chunk d. One batched 3D-AP DMA per slice.
            xk_q = [xsp.tile([128, ND * QC], F16, tag=f"xk{t}", name=f"xk{t}")
                    for t in range(NQ)]
            xq_q = [xsp.tile([128, ND * QC], F16, tag=f"xq{t}", name=f"xq{t}")
                    for t in range(NQ)]
            xv_q = [xsp.tile([128, ND * QC], F16, tag=f"xv{t}", name=f"xv{t}")
                    for t in range(NQ)]

            def dma_slice(dst, srcT, t):
                nc.sync.dma_start(
                    dst[:].rearrange("p (d c) -> p d c", d=ND),
                    srcT[:, t * QC:(t + 1) * QC]
                    .rearrange("(d p) c -> p d c", p=128))


            dma_slice(xk_q[0], xkT, 0)
            for p in range(2):
                nc.sync.dma_start(wk_sb[p][:], wk[p][:])
            dma_slice(xq_q[0], xqT, 0)
            for p in range(2):
                nc.sync.dma_start(wq_sb[p][:], wq[p][:])
            nc.sync.dma_start(wv_sb[:], wv[:])
            dma_slice(xv_q[0], xvT, 0)
            dma_slice(xk_q[1], xkT, 1)
            dma_slice(xv_q[1], xvT, 1)
            dma_slice(xk_q[2], xkT, 2)
            dma_slice(xv_q[2], xvT, 2)
            dma_slice(xk_q[3], xkT, 3)
            dma_slice(xq_q[1], xqT, 1)
            dma_slice(xv_q[3], xvT, 3)
            dma_slice(xq_q[2], xqT, 2)
            dma_slice(xq_q[3], xqT, 3)
            for p in range(2):
                nc.sync.dma_start(wo_sb[p][:], wo[p][:])

            # ---- persistent activations ----
            kt = [persist.tile([128, T], F16, tag=f"kt{p}", name=f"kt{p}") for p in range(2)]
            qt = [[persist.tile([128, QC], F16, tag=f"qt{p}_{t}", name=f"qt{p}_{t}")
                   for t in range(NQ)] for p in range(2)]
            # token-major [V|1] per key-block: cols idx*65..idx*65+63 = V of
            # head idx (=2p+h), col idx*65+64 = ones (fused row-sum)
            v2 = [persist.tile([128, 4 * 65], BF16, tag=f"v2_{b}", name=f"v2_{b}")
                  for b in range(NKB)]
            for b in range(NKB):
                for idx in range(4):
                    nc.vector.memset(v2[b][:, idx * 65 + 64: idx * 65 + 65], 1.0)

            # ---- projection emitters ----
            def proj_full(w_sb_p, x_q, dst_ap):
                """one [128, 512] projection slice in one go"""
                ps = psP.tile([128, QC], F32, tag="pp", name="proj_ps")
                for d in range(ND):
                    nc.tensor.matmul(ps[:], w_sb_p[:, d * 128:(d + 1) * 128],
                                     x_q[:, d * QC:(d + 1) * QC],
                                     start=(d == 0), stop=(d == ND - 1))
                nc.vector.tensor_copy(dst_ap, ps[:])

            def proj_chunks(w_sb_p, x_q, dst_ap, splits):
                """same, split into len(splits) PE bites for interleaving"""
                state = {}
                bnds = [0]
                for s in splits:
                    bnds.append(bnds[-1] + s)
                assert bnds[-1] == ND
                out = []
                for ci in range(len(splits)):
                    lo, hi = bnds[ci], bnds[ci + 1]

                    def f(lo=lo, hi=hi):
                        if "ps" not in state:
                            state["ps"] = psP.tile([128, QC], F32, tag="pp",
                                                   name="proj_ps")
                        ps = state["ps"]
                        for d in range(lo, hi):
                            nc.tensor.matmul(
                                ps[:], w_sb_p[:, d * 128:(d + 1) * 128],
                                x_q[:, d * QC:(d + 1) * QC],
                                start=(d == 0), stop=(d == ND - 1))
                        if hi == ND:
                            nc.vector.tensor_copy(dst_ap, ps[:])
                    out.append(f)
                return out

            def v_slice(b):
                """token-major V for key-block b: all 4 heads at once."""
                tq, c0 = b // 4, (b % 4) * 128
                pv = psP.tile([128, QC], F32, tag="pp", name=f"v_ps{b}")
                for d in range(ND):
                    nc.tensor.matmul(pv[:, 0:256],
                                     xv_q[tq][:, d * QC + c0: d * QC + c0 + 128],
                                     wv_sb[:, d * 256:(d + 1) * 256],
                                     start=(d == 0), stop=(d == ND - 1))
                for idx in range(4):
                    nc.vector.tensor_copy(
                        v2[b][:, idx * 65: idx * 65 + 64],
                        pv[:, idx * 64:(idx + 1) * 64])

            # PE warmup: garbage matmuls keep the HAM clock gate open
            # (K=8/8) until the first real projection
            for i in range(16):
                dmy = psO.tile([65, QC], F32, tag=f"o{i % 2}", name=f"wu_{i}")
                nc.tensor.matmul(dmy[:], junk[:, 512:577], junk[:, 0:QC],
                                 start=True, stop=True)

            # K t0 + Q t0 first: unlocks S^T for query chunk 0 ASAP
            proj_full(wk_sb[0], xk_q[0], kt[0][:, 0:QC])
            proj_full(wk_sb[1], xk_q[0], kt[1][:, 0:QC])
            proj_full(wq_sb[0], xq_q[0], qt[0][0][:])
            proj_full(wq_sb[1], xq_q[0], qt[1][0][:])

            # deferred projection work, interleaved into attention groups:
            # (qc, p) -> {kb: [thunk, ...]}
            hooks = {}

            def add_hook(qc, p, kb, fn):
                hooks.setdefault((qc, p), {}).setdefault(kb, []).append(fn)

            def add_chunked(qc, p, kb0, w_sb_p, x_q, dst_ap, splits=(4, 4)):
                for i, f in enumerate(
                        proj_chunks(w_sb_p, x_q, dst_ap, splits)):
                    add_hook(qc, p, kb0 + i, f)

            # V token-major: one key-block per kb of group (0,0)
            for b in range(NKB):
                add_hook(0, 0, b, (lambda b=b: v_slice(b)))
            # remaining K slices: p0 just before its S^T deadline in (0,0);
            # p1 early in (0,1) before ITS deadlines there
            for t in range(1, NQ):
                add_chunked(0, 0, 4 * t - 2, wk_sb[0], xk_q[t],
                            kt[0][:, t * QC:(t + 1) * QC])
                add_chunked(0, 1, 2 * t - 2, wk_sb[1], xk_q[t],
                            kt[1][:, t * QC:(t + 1) * QC])
            # remaining Q chunks: qt[t] needed at group (t, *)
            add_chunked(0, 1, 8, wq_sb[0], xq_q[1], qt[0][1][:], (3, 3, 2))
            add_chunked(0, 1, 11, wq_sb[1], xq_q[1], qt[1][1][:], (3, 3, 2))
            for t in (2, 3):
                g = (t - 1, 0)
                add_chunked(g[0], g[1], 1, wq_sb[0], xq_q[t], qt[0][t][:],
                            (3, 3, 2))
                add_chunked(g[0], g[1], 4, wq_sb[1], xq_q[t], qt[1][t][:],
                            (3, 3, 2))

            # ---- attention + output projection ----
            ost_live = {}

            def emit_outproj_group(qc, o2t, sub, oc, anchor):
                q0 = qc * QC
                if oc == 0:
                    ost_live[(qc, sub)] = osp.tile(
                        [128, D], F16, tag="os", name=f"os_{qc}_{sub}")
                ost = ost_live[(qc, sub)]
                pp = psP.tile([128, QC], F32, tag="pp", name=f"pp_{qc}_{sub}_{oc}")
                for p in range(2):
                    mm = nc.tensor.matmul(
                        pp[:],
                        o2t[p][:, sub * 128:(sub + 1) * 128],
                        wo_sb[p][:, oc * QC:(oc + 1) * QC],
                        start=(p == 0), stop=(p == 1))
                    if p == 0 and anchor is not None:
                        add_dep_helper(mm.ins, anchor.ins, sync=False,
                                       reason="interleave outproj after S")
                nc.vector.tensor_copy(
                    ost[:, oc * QC:(oc + 1) * QC], pp[:])
                if oc == 1:
                    nc.sync.dma_start(
                        pout[q0 + sub * 128:q0 + (sub + 1) * 128, :],
                        ost[:])
                    del ost_live[(qc, sub)]

            # last-chunk p0-half of the output projection, precomputed into
            # SBUF during the (3,1) stream so only p1 remains in the tail
            s3 = {}

            def emit_outproj3_p0(o2t_p0, sub, oc, anchor):
                s3[(sub, oc)] = persist.tile([128, QC], F16,
                                             tag=f"s3_{sub}_{oc}",
                                             name=f"s3_{sub}_{oc}")
                pp = psP.tile([128, QC], F32, tag="pp", name=f"p3_{sub}_{oc}")
                mm = nc.tensor.matmul(pp[:],
                                      o2t_p0[:, sub * 128:(sub + 1) * 128],
                                      wo_sb[0][:, oc * QC:(oc + 1) * QC],
                                      start=True, stop=True)
                if anchor is not None:
                    add_dep_helper(mm.ins, anchor.ins, sync=False,
                                   reason="interleave outproj3 after S")
                nc.vector.tensor_copy(s3[(sub, oc)][:], pp[:])

            def emit_outproj3_p1(o2t_p1):
                for sub in range(4):
                    ost = osp.tile([128, D], F16, tag="os", name=f"os3_{sub}")
                    for oc in range(2):
                        pp = psP.tile([128, QC], F32, tag="pp",
                                      name=f"pf_{sub}_{oc}")
                        nc.tensor.matmul(pp[:],
                                         o2t_p1[:, sub * 128:(sub + 1) * 128],
                                         wo_sb[1][:, oc * QC:(oc + 1) * QC],
                                         start=True, stop=True)
                        nc.vector.tensor_tensor(
                            ost[:, oc * QC:(oc + 1) * QC],
                            s3[(sub, oc)][:], pp[:],
                            mybir.AluOpType.add)
                    nc.sync.dma_start(
                        pout[3 * QC + sub * 128:3 * QC + (sub + 1) * 128, :],
                        ost[:])

            pending = None
            for qc in range(NQ):
                o2t = [o2tp.tile([128, QC], F16, tag=f"o2t{p}", name=f"o2t_{qc}_{p}")
                       for p in range(2)]
                for p in range(2):
                    gh = hooks.get((qc, p), {})
                    po = [psO.tile([65, QC], F32, tag=f"o{h}", name=f"po_{qc}_{p}_{h}")
                          for h in range(2)]
                    for kb in range(NKB):
                        k0 = kb * KB
                        ps = psS.tile([128, 2 * QC], F32, tag="s", name=f"s_{qc}_{p}_{kb}")
                        s_anchor = nc.tensor.matmul(
                            ps[:, 0:QC],
                            kt[p][0:64, k0:k0 + KB],
                            qt[p][qc][0:64, :],
                            start=True, stop=True, tile_position=(0, 0))
                        nc.tensor.matmul(
                            ps[:, QC:2 * QC],
                            kt[p][64:128, k0:k0 + KB],
                            qt[p][qc][64:128, :],
                            start=True, stop=True, tile_position=(64, 0))
                        for fn in gh.get(kb, ()):
                            fn()
                        at = atp.tile([128, 2 * QC], BF16, tag="at", name=f"at_{qc}_{p}_{kb}")
                        nc.scalar.activation(
                            at[:], ps[:], mybir.ActivationFunctionType.Exp)
                        for h in range(2):
                            idx = 2 * p + h
                            nc.tensor.matmul(
                                po[h][:],
                                v2[kb][:, idx * 65: idx * 65 + 65],
                                at[:, h * QC:(h + 1) * QC],
                                start=(kb == 0), stop=(kb == NKB - 1))
                        if p == 0 and pending is not None and 8 <= kb < 16:
                            pqc, po2t = pending
                            emit_outproj_group(pqc, po2t, (kb - 8) // 2,
                                               (kb - 8) % 2, s_anchor)
                        if qc == 3 and p == 1 and kb >= 8:
                            emit_outproj3_p0(o2t[0], (kb - 8) // 2,
                                             (kb - 8) % 2, s_anchor)
                    # evacuate O^T fast first (frees PSUM for the next
                    # group's AV), then normalize off the critical path
                    ots = []
                    for h in range(2):
                        ot = smallp.tile([65, QC], F32, tag=f"ot{h}", name=f"ot_{qc}_{p}_{h}")
                        nc.vector.tensor_copy(ot[:], po[h][:])
                        ots.append(ot)
                    rsl = []
                    for h in range(2):
                        r = smallp.tile([1, QC], F32, tag=f"r{h}", name=f"r_{qc}_{p}_{h}")
                        if qc == 3 and p == 1 and h == 1:
                            # no more exps -> ACT is idle; 1/d = exp(-ln d)
                            # runs in parallel with h0's DVE reciprocal
                            lg = smallp.tile([1, QC], F32, tag=f"lg{h}",
                                             name=f"lg_{h}")
                            nc.scalar.activation(
                                lg[:], ots[h][64:65, :],
                                mybir.ActivationFunctionType.Ln)
                            nc.scalar.activation(
                                r[:], lg[:],
                                mybir.ActivationFunctionType.Exp,
                                scale=-1.0)
                        else:
                            nc.vector.reciprocal(r[:], ots[h][64:65, :])
                        rsl.append(r)
                    rbs = []
                    for h in range(2):
                        rb = smallp.tile([64, QC], F32, tag=f"rb{h}", name=f"rb_{qc}_{p}_{h}")
                        nc.gpsimd.partition_broadcast(rb[:], rsl[h][:])
                        rbs.append(rb)
                    for h in range(2):
                        nc.vector.tensor_mul(
                            o2t[p][h * 64:(h + 1) * 64, :],
                            ots[h][0:64, :], rbs[h][:])
                    if p == 1:
                        pending = None
                pending = (qc, o2t)

            # tail: PE warm-keeping dummies under the final normalize chain
            for i in range(36):
                dmy = psO.tile([65, QC], F32, tag=f"o{i % 2}", name=f"dmy_{i}")
                nc.tensor.matmul(dmy[:], kt[0][:, 0:65], kt[1][:, 0:QC],
                                 start=True, stop=True)
            emit_outproj3_p1(pending[1][1])

    nc.compile()
    nc.m = get_hw_module(nc.m)
    return nc


def _pack_w(w_pair):
    # w_pair: [2, 1024, 64] -> [1024, 128] -> chunk-major [128, 8*128]
    w = np.concatenate([w_pair[0], w_pair[1]], axis=1)          # [1024, 128]
    return np.ascontiguousarray(
        w.reshape(ND, 128, 128).transpose(1, 0, 2).reshape(128, D))


def _pack_wv(w4):
    # w4: [4, 1024, 64] -> [128, 8*256]; chunk d at cols d*256, head idx
    # (=2p+h) at idx*64 within the chunk
    return np.ascontiguousarray(
        w4.reshape(4, ND, 128, E).transpose(2, 1, 0, 3).reshape(128, 2 * D))


def _pack_wo(wo_pair):
    # wo_pair: [2, 64, 1024] -> [128, 1024]
    return np.ascontiguousarray(np.concatenate([wo_pair[0], wo_pair[1]], axis=0))


def kernel(q, k, v, W_query, W_key, W_val, W_out, _trace=False):
    q = np.asarray(q, dtype=np.float32)
    k = np.asarray(k, dtype=np.float32)
    v = np.asarray(v, dtype=np.float32)
    W_query = np.asarray(W_query, dtype=np.float32)
    W_key = np.asarray(W_key, dtype=np.float32)
    W_val = np.asarray(W_val, dtype=np.float32)
    W_out = np.asarray(W_out, dtype=np.float32)

    if "nc" not in _CACHE:
        _CACHE["nc"] = _build()
    nc = _CACHE["nc"]

    norm = 1.0 / np.sqrt(E)
    xT = {}
    for b in range(2):
        xT[("q", b)] = np.ascontiguousarray(q[b].T).astype(np.float16)
        xT[("k", b)] = np.ascontiguousarray(k[b].T).astype(np.float16)
        xT[("v", b)] = np.ascontiguousarray(v[b].T).astype(np.float16)

    in_maps = []
    for c in range(N_CORES):
        b, g = c // 4, c % 4
        hs = [4 * g, 4 * g + 1, 4 * g + 2, 4 * g + 3]
        m = {
            "xqT": xT[("q", b)], "xkT": xT[("k", b)], "xvT": xT[("v", b)],
            "wv": _pack_wv(W_val[hs]).astype(np.float16),
        }
        for p in range(2):
            hp = hs[2 * p:2 * p + 2]
            m[f"wq{p}"] = _pack_w(W_query[hp] * norm).astype(np.float16)
            m[f"wk{p}"] = _pack_w(W_key[hp]).astype(np.float16)
            m[f"wo{p}"] = _pack_wo(W_out[hp]).astype(np.float16)
        in_maps.append(m)

    res = run_bass_kernel_spmd(nc, in_maps, list(range(N_CORES)),
                               trace=_trace)
    parts = [res.results[c]["pout"].astype(np.float32) for c in range(N_CORES)]
    out = np.stack([
        parts[0] + parts[1] + parts[2] + parts[3],
        parts[4] + parts[5] + parts[6] + parts[7],
    ]).astype(np.float32)
    if _trace:
        _CACHE["last_result"] = res
    return out
